# revision 1
# baseline (speedup 1.0000x reference)
"""GQA attention + RoPE, tensor-parallel across 8 NeuronCores (Bass/Tile).

Model: x(1,2048,2048) -> Q=xWq (32 heads x 64), K/V=xWk/xWv (8 kv heads),
RoPE on q/k, causal softmax attention (GQA: 4 q heads per kv head), out-proj.

Sharding: head-parallel. Core i gets q heads 4i..4i+3 (Wq cols), kv head i
(Wk/Wv cols), Wo rows 256i..256i+256. Each core computes a partial (2048,2048)
output; host sums the 8 partials (the "all-reduce").

Per-core layout strategy (everything feature-on-partitions):
  xT (128,16,2048): x^T tiled by feature blocks; streamed in 512-seq chunks.
  Q^T/K^T computed directly via matmul(lhsT=W-tile, rhs=xT-chunk) -> (d, s).
  RoPE: out = q*C + swap32(q)*S with host-replicated trig tables; the
  half-swap is done with small SBUF<-PSUM DMAs (engines are partition-locked).
  Scores computed transposed: ST[t,s] = K^T-block^T @ Q^T-chunk, k=64, with
  the two heads of a pair row-packed into PE row groups 0-63/64-127.
  Softmax without max-subtraction (scores are O(5), fp32 exp is safe):
  exp on ACT fused with the 1/8 scale; causal handled by a triangular mask
  multiply on diagonal 128-blocks plus memset of fully-masked columns.
  ctx^T = V'-block^T @ expST with V' = [V | ones] so psum row 64 accumulates
  the softmax denominator for free; normalize = reciprocal + gpsimd
  partition_broadcast + multiply.
  Out-proj: psum += ctxT-tile^T @ Wo-rows, k=128 over 2 head-pair tiles.
All matmuls run as float32r (full PE rate at moving-dim>=256).
"""

import numpy as np
from contextlib import ExitStack

import concourse.bass as bass
from concourse import bacc
import concourse.tile as tile
from concourse import mybir
from concourse.bass_utils import run_bass_kernel_spmd

F32 = mybir.dt.float32
F32R = mybir.dt.float32r
AF = mybir.ActivationFunctionType

S = 2048          # sequence length
D = 2048          # model dim
HD = 64           # head dim
NCORES = 8
QH = 4            # q heads per core
QC = QH * HD      # 256 q columns per core
SC = 512          # seq chunk width
NSC = S // SC     # 4 chunks
KB = D // 128     # 16 feature blocks
SCALE = 1.0 / 8.0  # 1/sqrt(64)

_NC = None
DEBUG = False


def _build():
    nc = bacc.Bacc(None)
    xT = nc.declare_dram_parameter("xT", [128, KB, S], F32R, isOutput=False)
    wq = nc.declare_dram_parameter("wq", [128, KB, QC], F32R, isOutput=False)
    wkv = nc.declare_dram_parameter("wkv", [128, KB, 128], F32R, isOutput=False)
    wo = nc.declare_dram_parameter("wo", [128, 2, D], F32R, isOutput=False)
    ctab = nc.declare_dram_parameter("ctab", [128, S], F32, isOutput=False)
    stab = nc.declare_dram_parameter("stab", [128, S], F32, isOutput=False)
    trimask = nc.declare_dram_parameter("trimask", [128, 128], F32, isOutput=False)
    eye = nc.declare_dram_parameter("eye", [64, 64], F32, isOutput=False)
    vones = nc.declare_dram_parameter("vones", [128, KB, 2], F32R, isOutput=False)
    zpad = nc.declare_dram_parameter("zpad", [128, 384], F32R, isOutput=False)
    out = nc.declare_dram_parameter("out", [S, D], F32, isOutput=True)
    if DEBUG:
        dq0 = nc.declare_dram_parameter("dq0", [128, S], F32, isOutput=True)
        dkt = nc.declare_dram_parameter("dkt", [128, S], F32, isOutput=True)
        dv = nc.declare_dram_parameter("dv", [128, KB, HD + 2], F32, isOutput=True)
        dc0 = nc.declare_dram_parameter("dc0", [128, S], F32, isOutput=True)
        dctxr = nc.declare_dram_parameter("dctxr", [HD + 2, SC], F32, isOutput=True)
        dbc = nc.declare_dram_parameter("dbc", [64, SC], F32, isOutput=True)
        dex = nc.declare_dram_parameter("dex", [128, SC], F32, isOutput=True)

    with tile.TileContext(nc) as tc, ExitStack() as ctx:
        sb = ctx.enter_context(tc.tile_pool(name="sb", bufs=1))
        xp = ctx.enter_context(tc.tile_pool(name="xp", bufs=16))
        wk_ = ctx.enter_context(tc.tile_pool(name="wk", bufs=2))
        pp = ctx.enter_context(tc.tile_pool(name="pp", bufs=2, space="PSUM"))

        # ---- persistent constants ----
        wq_sb = sb.tile([128, KB, QC], F32R)
        nc.sync.dma_start(out=wq_sb, in_=wq[:, :, :])
        wkv_sb = sb.tile([128, KB, 128], F32R)
        nc.sync.dma_start(out=wkv_sb, in_=wkv[:, :, :])
        wo_sb = sb.tile([128, 2, D], F32R)
        nc.sync.dma_start(out=wo_sb, in_=wo[:, :, :])
        ctab_sb = sb.tile([128, S], F32)
        nc.sync.dma_start(out=ctab_sb, in_=ctab[:, :])
        stab_sb = sb.tile([128, S], F32)
        nc.sync.dma_start(out=stab_sb, in_=stab[:, :])
        tri_sb = sb.tile([128, 128], F32)
        nc.sync.dma_start(out=tri_sb, in_=trimask[:, :])
        eye_sb = sb.tile([64, 64], F32)
        nc.sync.dma_start(out=eye_sb, in_=eye[:, :])

        # ---- persistent activations ----
        qt0 = sb.tile([128, S], F32R)   # q^T heads 0,1 (roped)
        qt1 = sb.tile([128, S], F32R)   # q^T heads 2,3
        qts = [qt0, qt1]
        kt_sb = sb.tile([128, S], F32R)  # rows 0-63 k^T roped; 64-127 duplicate
        v_sb = sb.tile([128, KB, HD + 2], F32R)  # V natural (t,d) + ones col
        ct0 = sb.tile([128, S], F32R)   # normalized ctx^T heads 0,1
        ct1 = sb.tile([128, S], F32R)
        cts = [ct0, ct1]
        nc.sync.dma_start(out=v_sb[:, :, HD:HD + 2], in_=vones[:, :, :])
        zpad_sb = sb.tile([128, 384], F32R)
        nc.sync.dma_start(out=zpad_sb, in_=zpad[:, :])

        def emit_proj(c):
            cs = slice(c * SC, (c + 1) * SC)
            xc = []
            for kb in range(KB):
                t = xp.tile([128, SC], F32R, name=f"xc_{c}_{kb}", tag="xc")
                nc.sync.dma_start(out=t, in_=xT[:, kb, cs])
                xc.append(t)
            # Q projection + rope, one 128-tile (2 heads) at a time
            for u in range(2):
                pq = pp.tile([128, SC], F32, name=f"pq_{c}_{u}", tag="pq")
                for kb in range(KB):
                    nc.tensor.matmul(
                        pq,
                        lhsT=wq_sb[:, kb, u * 128:(u + 1) * 128],
                        rhs=xc[kb],
                        start=(kb == 0), stop=(kb == KB - 1),
                    )
                qraw = wk_.tile([128, SC], F32, name=f"qraw_{c}_{u}", tag="qraw")
                nc.scalar.copy(qraw, pq)
                t1 = wk_.tile([128, SC], F32, name=f"rt1_{c}_{u}", tag="rt", bufs=2)
                nc.vector.tensor_mul(t1, pq, ctab_sb[:, cs])
                qsw = wk_.tile([128, SC], F32, name=f"qsw_{c}_{u}", tag="qsw")
                for b in (0, 64):
                    nc.sync.dma_start(out=qsw[b:b + 32, :], in_=qraw[b + 32:b + 64, :])
                    nc.sync.dma_start(out=qsw[b + 32:b + 64, :], in_=qraw[b:b + 32, :])
                t2 = wk_.tile([128, SC], F32, name=f"rt2_{c}_{u}", tag="rt", bufs=2)
                nc.vector.tensor_mul(t2, qsw, stab_sb[:, cs])
                nc.vector.tensor_add(qts[u][:, cs], t1, t2)
            # KV projection
            pkv = pp.tile([128, SC], F32, name=f"pkv_{c}", tag="pq")
            for kb in range(KB):
                nc.tensor.matmul(
                    pkv,
                    lhsT=wkv_sb[:, kb, :],
                    rhs=xc[kb],
                    start=(kb == 0), stop=(kb == KB - 1),
                )
            # K rope (rows 0-63); V raw (rows 64-127) to sbuf
            kvraw = wk_.tile([128, SC], F32, name=f"kvraw_{c}", tag="qraw")
            nc.scalar.copy(kvraw, pkv)
            k1 = wk_.tile([64, SC], F32, name=f"kr1_{c}", tag="krt", bufs=2)
            nc.vector.tensor_mul(k1, pkv[0:64, :], ctab_sb[0:64, cs])
            ksw = wk_.tile([64, SC], F32, name=f"ksw_{c}", tag="ksw")
            nc.sync.dma_start(out=ksw[0:32, :], in_=kvraw[32:64, :])
            nc.sync.dma_start(out=ksw[32:64, :], in_=kvraw[0:32, :])
            k2 = wk_.tile([64, SC], F32, name=f"kr2_{c}", tag="krt", bufs=2)
            nc.vector.tensor_mul(k2, ksw, stab_sb[0:64, cs])
            nc.vector.tensor_add(kt_sb[0:64, cs], k1, k2)
            nc.sync.dma_start(out=kt_sb[64:128, cs], in_=kt_sb[0:64, cs])
            # V natural layout: shift rows 64-127 down, then PE-transpose
            # each 128-seq block
            vtr = wk_.tile([64, SC], F32, name=f"vtr_{c}", tag="vtr")
            nc.sync.dma_start(out=vtr, in_=kvraw[64:128, :])
            for r in range(4):
                j = 4 * c + r
                pt = pp.tile([128, HD], F32, name=f"pt_{c}_{r}", tag="ps")
                nc.tensor.transpose(pt, vtr[:, r * 128:(r + 1) * 128], eye_sb)
                nc.any.tensor_copy(v_sb[:, j, 0:HD], pt)

        def emit_attn(c):
            cs = slice(c * SC, (c + 1) * SC)
            for u in range(2):
                cps = []
                for idx in range(2):
                    cpt = pp.tile([HD + 2, SC], F32, name=f"cp_{c}_{u}_{idx}",
                                  tag="pc")
                    cps.append(cpt)
                njt = 4 * c + 4
                for j in range(njt):
                    diag = j >= 4 * c
                    r = j - 4 * c
                    jb = slice(j * 128, (j + 1) * 128)
                    # columns of this chunk still unmasked for t-block j
                    lo = 128 * r if diag else 0
                    nsl = slice(lo, SC)
                    csl = slice(c * SC + lo, (c + 1) * SC)
                    es = []
                    for idx in range(2):
                        sp = pp.tile([128, SC], F32, name=f"sp_{c}_{u}_{j}_{idx}",
                                     tag="ps")
                        nc.tensor.matmul(
                            sp[:, nsl],
                            lhsT=kt_sb[idx * 64:idx * 64 + 64, jb],
                            rhs=qts[u][idx * 64:idx * 64 + 64, csl],
                            start=True, stop=True,
                            tile_position=(idx * 64, 0),
                        )
                        e = wk_.tile([128, SC], F32R, name=f"e_{c}_{u}_{j}_{idx}",
                                     tag="ex", bufs=3)
                        if lo:
                            nc.sync.dma_start(out=e[:, 0:lo], in_=zpad_sb[:, 0:lo])
                        nc.scalar.activation(e[:, nsl], sp[:, nsl], AF.Exp,
                                             scale=SCALE)
                        if diag:
                            dsl = slice(lo, lo + 128)
                            nc.vector.tensor_mul(e[:, dsl], e[:, dsl], tri_sb)
                        es.append(e)
                    for idx in range(2):
                        nc.tensor.matmul(
                            cps[idx],
                            lhsT=v_sb[:, j, :],
                            rhs=es[idx],
                            start=(j == 0), stop=(j == njt - 1),
                        )
                    if DEBUG and c == 0 and u == 0 and j == 0:
                        dxt = wk_.tile([128, SC], F32, name="dxt", tag="dxt")
                        nc.vector.tensor_copy(dxt, es[0])
                        nc.sync.dma_start(out=dex[:, :], in_=dxt)
                if DEBUG and c == 0 and u == 0:
                    dcc = wk_.tile([HD + 2, SC], F32, name="dcc", tag="dcc")
                    nc.vector.tensor_copy(dcc, cps[0])
                    nc.sync.dma_start(out=dctxr[:, :], in_=dcc)
                for idx in range(2):
                    cpy = wk_.tile([HD + 2, SC], F32, name=f"cpy_{c}_{u}_{idx}",
                                   tag="cpy")
                    nc.any.tensor_copy(cpy, cps[idx])
                    den0 = wk_.tile([1, SC], F32, name=f"den_{c}_{u}_{idx}",
                                    tag="den")
                    nc.sync.dma_start(out=den0, in_=cpy[HD:HD + 1, :])
                    rec0 = wk_.tile([1, SC], F32, name=f"rec_{c}_{u}_{idx}",
                                    tag="rec")
                    nc.vector.reciprocal(rec0, den0)
                    bc = wk_.tile([64, SC], F32, name=f"bc_{c}_{u}_{idx}",
                                  tag="bc")
                    nc.gpsimd.partition_broadcast(bc, rec0)
                    if DEBUG and c == 0 and u == 0 and idx == 0:
                        nc.sync.dma_start(out=dbc[:, :], in_=bc)
                    if idx == 0:
                        nc.vector.tensor_mul(cts[u][0:64, cs], cpy[0:64, :], bc)
                    else:
                        ns = wk_.tile([64, SC], F32R, name=f"ns_{c}_{u}", tag="ns")
                        nc.vector.tensor_mul(ns, cpy[0:64, :], bc)
                        nc.sync.dma_start(out=cts[u][64:128, cs], in_=ns)

        def emit_outproj(c):
            for mi in range(4):
                m = 4 * c + mi
                mb = slice(m * 128, (m + 1) * 128)
                for n in range(4):
                    nck = slice(n * SC, (n + 1) * SC)
                    po = pp.tile([128, SC], F32, name=f"po_{c}_{mi}_{n}", tag="po")
                    for u in range(2):
                        nc.tensor.matmul(
                            po,
                            lhsT=cts[u][:, mb],
                            rhs=wo_sb[:, u, nck],
                            start=(u == 0), stop=(u == 1),
                        )
                    ob = wk_.tile([128, SC], F32, name=f"ob_{c}_{mi}_{n}",
                                  tag="ob", bufs=2)
                    nc.vector.tensor_copy(ob, po)
                    nc.sync.dma_start(out=out[mb, nck], in_=ob)

        emit_proj(0)
        for c in range(NSC):
            if c + 1 < NSC:
                emit_proj(c + 1)
            emit_attn(c)
            emit_outproj(c)
        if DEBUG:
            nc.sync.dma_start(out=dq0[:, :], in_=qt0.bitcast(F32))
            nc.sync.dma_start(out=dkt[:, :], in_=kt_sb.bitcast(F32))
            nc.sync.dma_start(out=dv[:, :, :], in_=v_sb.bitcast(F32))
            nc.sync.dma_start(out=dc0[:, :], in_=ct0.bitcast(F32))

    nc.finalize()
    return nc


def _get_nc():
    global _NC
    if _NC is None:
        _NC = _build()
    return _NC


def _prep_in_maps(x, Wq, Wk, Wv, Wo, cos, sin):
    x0 = np.ascontiguousarray(np.asarray(x, np.float32).reshape(S, D))
    xT = np.ascontiguousarray(x0.T.reshape(KB, 128, S).transpose(1, 0, 2))
    cosT = np.ascontiguousarray(np.asarray(cos, np.float32).T)  # (32, S)
    sinT = np.ascontiguousarray(np.asarray(sin, np.float32).T)
    ctab = np.tile(cosT, (4, 1))                                   # (128, S)
    stab = np.tile(np.vstack([-sinT, sinT]), (2, 1))               # (128, S)
    trimask = (np.arange(128)[:, None] <= np.arange(128)[None, :]).astype(
        np.float32)
    eye = np.eye(64, dtype=np.float32)
    vones_a = np.zeros((128, KB, 2), np.float32); vones_a[:, :, 0] = 1.0
    zpad_a = np.zeros((128, 384), np.float32)
    Wq = np.asarray(Wq, np.float32)
    Wk = np.asarray(Wk, np.float32)
    Wv = np.asarray(Wv, np.float32)
    Wo = np.asarray(Wo, np.float32)

    in_maps = []
    for i in range(NCORES):
        wq_i = np.ascontiguousarray(
            Wq[:, i * QC:(i + 1) * QC].reshape(KB, 128, QC).transpose(1, 0, 2))
        wkv_i = np.concatenate(
            [Wk[:, i * HD:(i + 1) * HD], Wv[:, i * HD:(i + 1) * HD]], axis=1)
        wkv_i = np.ascontiguousarray(
            wkv_i.reshape(KB, 128, 128).transpose(1, 0, 2))
        wo_i = np.ascontiguousarray(
            Wo[i * QC:(i + 1) * QC, :].reshape(2, 128, D).transpose(1, 0, 2))
        in_maps.append({
            "xT": xT, "wq": wq_i, "wkv": wkv_i, "wo": wo_i,
            "ctab": ctab, "stab": stab, "trimask": trimask, "eye": eye,
            "vones": vones_a, "zpad": zpad_a,
        })
    return in_maps


def run(inputs, **kw):
    nc = _get_nc()
    in_maps = _prep_in_maps(**inputs)
    return run_bass_kernel_spmd(nc, in_maps, list(range(NCORES)), **kw)


def kernel(x, Wq, Wk, Wv, Wo, cos, sin):
    res = run(dict(x=x, Wq=Wq, Wk=Wk, Wv=Wv, Wo=Wo, cos=cos, sin=sin))
    acc = np.zeros((S, D), np.float32)
    for r in res.results:
        acc += r["out"]
    return acc.reshape(1, S, D)



# revision 12
# speedup vs baseline: 1.5797x; 1.5797x over previous
"""GQA attention + RoPE, tensor-parallel across 8 NeuronCores (Bass/Tile).

Model: x(1,2048,2048) -> Q=xWq (32 heads x 64), K/V=xWk/xWv (8 kv heads),
RoPE on q/k, causal softmax attention (GQA: 4 q heads per kv head), out-proj.

Sharding: head-parallel. Core i gets q heads 4i..4i+3 (Wq cols), kv head i
(Wk/Wv cols), Wo rows 256i..256i+256. Each core computes a partial (2048,2048)
output; host sums the 8 partials (the "all-reduce").

v2 changes vs v1 (427us baseline):
  * All matmul operands bf16 (psum accum stays fp32): same PE rate as f32r
    but FWL halves LDWEIGHTS, DMA bytes halve, DVE gets 2x/4x modes.
  * RoPE without the ACT pre-copy: out = q*C + swap32(q*S2) with S2 a
    pre-swapped sign-adjusted sin table, so the DMA half-swap reads the
    DVE product directly (psum -> 2 DVE muls -> 4 small DMAs -> add).
  * Softmax normalize: reciprocal_approx_fast directly on the psum den row
    (was: 1-partition DMA + 3.3us DVE reciprocal), gpsimd broadcast from
    partition 64, single mul. No den DMA.
  * Causal handling: ctx matmuls restrict rhs/out to the live column range
    instead of zero-padding e (drops the zpad DMAs).
  * xc streamed as 4 DMAs of 4 k-blocks; emission order attn(c), proj(c+1),
    outproj(c) so ready proj/outproj matmuls fill PE stalls.
"""

import numpy as np
from contextlib import ExitStack

import concourse.bass as bass
from concourse import bacc
import concourse.tile as tile
from concourse import mybir
from concourse.bass_utils import run_bass_kernel_spmd

F32 = mybir.dt.float32
BF = mybir.dt.bfloat16
AF = mybir.ActivationFunctionType

S = 2048          # sequence length
D = 2048          # model dim
HD = 64           # head dim
NCORES = 8
QH = 4            # q heads per core
QC = QH * HD      # 256 q columns per core
SC = 512          # seq chunk width
NSC = S // SC     # 4 chunks
KB = D // 128     # 16 feature blocks
SCALE = 1.0 / 8.0  # 1/sqrt(64)

_NC = None


def _build():
    nc = bacc.Bacc(None)
    xT = nc.declare_dram_parameter("xT", [128, KB, S], BF, isOutput=False)
    wq = nc.declare_dram_parameter("wq", [128, KB, QC], BF, isOutput=False)
    wkv = nc.declare_dram_parameter("wkv", [128, KB, 128], BF, isOutput=False)
    wo = nc.declare_dram_parameter("wo", [128, 2, D], BF, isOutput=False)
    ctab = nc.declare_dram_parameter("ctab", [128, S], BF, isOutput=False)
    stab2 = nc.declare_dram_parameter("stab2", [128, S], BF, isOutput=False)
    trimask = nc.declare_dram_parameter("trimask", [128, 128], BF, isOutput=False)
    eye = nc.declare_dram_parameter("eye", [64, 64], F32, isOutput=False)
    vones = nc.declare_dram_parameter("vones", [128, KB, 2], BF, isOutput=False)
    vpad = nc.declare_dram_parameter("vpad", [128, KB, 63], BF, isOutput=False)
    out = nc.declare_dram_parameter("out", [S, D], BF, isOutput=True)

    with tile.TileContext(nc) as tc, ExitStack() as ctx:
        sb = ctx.enter_context(tc.tile_pool(name="sb", bufs=1))
        xp = ctx.enter_context(tc.tile_pool(name="xp", bufs=2))
        wk_ = ctx.enter_context(tc.tile_pool(name="wk", bufs=2))
        pp = ctx.enter_context(tc.tile_pool(name="pp", bufs=2, space="PSUM"))

        # ---- persistent constants ----
        wq_sb = sb.tile([128, KB, QC], BF)
        nc.sync.dma_start(out=wq_sb, in_=wq[:, :, :])
        wkv_sb = sb.tile([128, KB, 128], BF)
        nc.sync.dma_start(out=wkv_sb, in_=wkv[:, :, :])
        ctab_sb = sb.tile([128, S], BF)
        nc.sync.dma_start(out=ctab_sb, in_=ctab[:, :])
        stab_sb = sb.tile([128, S], BF)
        nc.sync.dma_start(out=stab_sb, in_=stab2[:, :])
        tri_sb = sb.tile([128, 128], BF)
        nc.sync.dma_start(out=tri_sb, in_=trimask[:, :])
        eye_sb = sb.tile([64, 64], F32)
        nc.sync.dma_start(out=eye_sb, in_=eye[:, :])
        wo_sb = sb.tile([128, 2, D], BF)
        nc.sync.dma_start(out=wo_sb, in_=wo[:, :, :])

        # ---- persistent activations ----
        qt0 = sb.tile([128, S], BF)   # q^T heads 0,1 (roped)
        qt1 = sb.tile([128, S], BF)   # q^T heads 2,3
        qts = [qt0, qt1]
        kt_sb = sb.tile([128, S], BF)  # rows 0-63 k^T roped; 64-127 duplicate
        v_sb = sb.tile([128, KB, HD + 2], BF)  # [V | ones | pad] (for idx1)
        v_sb2 = sb.tile([128, KB, 128], BF)    # [ones | zeros*63 | V] (idx0)
        ct0 = sb.tile([128, S], BF)   # normalized ctx^T: rows 0-63 head 2u+1,
        ct1 = sb.tile([128, S], BF)   # rows 64-127 head 2u (wo host-reordered)
        cts = [ct0, ct1]
        nc.sync.dma_start(out=v_sb[:, :, HD:HD + 2], in_=vones[:, :, :])
        nc.sync.dma_start(out=v_sb2[:, :, 0:1], in_=vones[:, :, 0:1])
        nc.sync.dma_start(out=v_sb2[:, :, 1:64], in_=vpad[:, :, :])

        def emit_proj(c):
            cs = slice(c * SC, (c + 1) * SC)
            xc = xp.tile([128, KB, SC], BF, name=f"xc_{c}", tag="xc")
            for g in range(4):
                nc.sync.dma_start(out=xc[:, 4 * g:4 * g + 4, :],
                                  in_=xT[:, 4 * g:4 * g + 4, cs])
            # Q projection + rope, one 128-tile (2 heads) at a time
            for u in range(2):
                pq = pp.tile([128, SC], F32, name=f"pq_{c}_{u}", tag="pq")
                for kb in range(KB):
                    nc.tensor.matmul(
                        pq,
                        lhsT=wq_sb[:, kb, u * 128:(u + 1) * 128],
                        rhs=xc[:, kb, :],
                        start=(kb == 0), stop=(kb == KB - 1),
                    )
                t1 = wk_.tile([128, SC], BF, name=f"rt1_{c}_{u}", tag="rt",
                              bufs=2)
                nc.vector.tensor_mul(t1, pq, ctab_sb[:, cs])
                w = wk_.tile([128, SC], BF, name=f"rw_{c}_{u}", tag="rw",
                             bufs=2)
                nc.vector.tensor_mul(w, pq, stab_sb[:, cs])
                wsw = wk_.tile([128, SC], BF, name=f"wsw_{c}_{u}", tag="wsw",
                               bufs=2)
                for b in (0, 64):
                    nc.sync.dma_start(out=wsw[b:b + 32, :],
                                      in_=w[b + 32:b + 64, :])
                    nc.sync.dma_start(out=wsw[b + 32:b + 64, :],
                                      in_=w[b:b + 32, :])
                nc.vector.tensor_add(qts[u][:, cs], t1, wsw)
            # KV projection
            pkv = pp.tile([128, SC], F32, name=f"pkv_{c}", tag="pq")
            for kb in range(KB):
                nc.tensor.matmul(
                    pkv,
                    lhsT=wkv_sb[:, kb, :],
                    rhs=xc[:, kb, :],
                    start=(kb == 0), stop=(kb == KB - 1),
                )
            # K rope (rows 0-63)
            k1 = wk_.tile([64, SC], BF, name=f"kr1_{c}", tag="krt", bufs=2)
            nc.vector.tensor_mul(k1, pkv[0:64, :], ctab_sb[0:64, cs])
            kw = wk_.tile([64, SC], BF, name=f"krw_{c}", tag="krw", bufs=2)
            nc.vector.tensor_mul(kw, pkv[0:64, :], stab_sb[0:64, cs])
            ksw = wk_.tile([64, SC], BF, name=f"ksw_{c}", tag="ksw")
            nc.sync.dma_start(out=ksw[0:32, :], in_=kw[32:64, :])
            nc.sync.dma_start(out=ksw[32:64, :], in_=kw[0:32, :])
            nc.vector.tensor_add(kt_sb[0:64, cs], k1, ksw)
            nc.sync.dma_start(out=kt_sb[64:128, cs], in_=kt_sb[0:64, cs])
            # V natural layout: copy psum rows 64-127 (fp32), shift down via
            # DMA, then PE-transpose each 128-seq block
            vraw = wk_.tile([128, SC], F32, name=f"vraw_{c}", tag="vraw")
            nc.vector.tensor_copy(vraw[64:128, :], pkv[64:128, :])
            vtr = wk_.tile([64, SC], F32, name=f"vtr_{c}", tag="vtr")
            nc.sync.dma_start(out=vtr, in_=vraw[64:128, :])
            for r in range(4):
                j = 4 * c + r
                pt = pp.tile([128, HD], F32, name=f"pt_{c}_{r}", tag="ps")
                nc.tensor.transpose(pt, vtr[:, r * 128:(r + 1) * 128], eye_sb)
                nc.vector.tensor_copy(v_sb[:, j, 0:HD], pt)
                nc.vector.tensor_copy(v_sb2[:, j, HD:2 * HD], pt)

        def emit_attn(c):
            cs = slice(c * SC, (c + 1) * SC)
            for u in range(2):
                # idx0: lhsT=[ones|0*63|V] -> den row 0, ctx rows 64-127
                # idx1: lhsT=[V|ones|pad]  -> ctx rows 0-63, den row 64
                cps = [
                    pp.tile([128, SC], F32, name=f"cp_{c}_{u}_0", tag="pc"),
                    pp.tile([HD + 2, SC], F32, name=f"cp_{c}_{u}_1", tag="pc"),
                ]
                njt = 4 * c + 4
                for j in range(njt):
                    diag = j >= 4 * c
                    r = j - 4 * c
                    jb = slice(j * 128, (j + 1) * 128)
                    # columns of this chunk still unmasked for t-block j
                    lo = 128 * r if diag else 0
                    nsl = slice(lo, SC)
                    csl = slice(c * SC + lo, (c + 1) * SC)
                    for idx in range(2):
                        sp = pp.tile([128, SC], F32, name=f"sp_{c}_{u}_{j}_{idx}",
                                     tag="ps")
                        nc.tensor.matmul(
                            sp[:, nsl],
                            lhsT=kt_sb[idx * 64:idx * 64 + 64, jb],
                            rhs=qts[u][idx * 64:idx * 64 + 64, csl],
                            start=True, stop=True,
                            tile_position=(idx * 64, 0),
                        )
                        e = wk_.tile([128, SC], BF, name=f"e_{c}_{u}_{j}_{idx}",
                                     tag="ex", bufs=3)
                        nc.scalar.activation(e[:, nsl], sp[:, nsl], AF.Exp,
                                             scale=SCALE)
                        if diag:
                            dsl = slice(lo, lo + 128)
                            nc.vector.tensor_mul(e[:, dsl], e[:, dsl], tri_sb)
                        nc.tensor.matmul(
                            cps[idx][:, nsl],
                            lhsT=(v_sb2[:, j, :] if idx == 0
                                  else v_sb[:, j, :]),
                            rhs=e[:, nsl],
                            start=(j == 0), stop=(j == njt - 1),
                            skip_group_check=True,
                        )
                # idx0: den at psum row 0 -> reciprocal -> broadcast (from
                # partition 0) to all 128 partitions; ctx rows 64-127 scale
                # in place into cts rows 64-127. No cross-partition DMA.
                rec0 = wk_.tile([1, SC], F32, name=f"rec0_{c}_{u}",
                                tag="rec0", bufs=2)
                nc.vector.reciprocal_approx_fast(rec0, cps[0][0:1, :])
                cnv0 = wk_.tile([128, SC], BF, name=f"cnv0_{c}_{u}",
                                tag="cnv0", bufs=2)
                nc.vector.tensor_copy(cnv0[64:128, :], cps[0][64:128, :])
                bc0 = wk_.tile([128, SC], F32, name=f"bc0_{c}_{u}",
                               tag="bc0", bufs=2)
                nc.gpsimd.partition_broadcast(bc0, rec0)
                nc.vector.tensor_mul(cts[u][64:128, cs], cnv0[64:128, :],
                                     bc0[64:128, :])
                # idx1: den at psum row 64 -> copy to sbuf, DMA to partition
                # 0, reciprocal there, broadcast, scale rows 0-63.
                dcp = wk_.tile([66, SC], F32, name=f"dcp_{c}_{u}",
                               tag="dcp", bufs=2)
                nc.vector.tensor_copy(dcp[64:65, :], cps[1][64:65, :])
                den0 = wk_.tile([1, SC], F32, name=f"den0_{c}_{u}",
                                tag="den0", bufs=2)
                nc.sync.dma_start(out=den0, in_=dcp[64:65, :])
                rec1 = wk_.tile([1, SC], F32, name=f"rec1_{c}_{u}",
                                tag="rec1", bufs=2)
                nc.vector.reciprocal_approx_fast(rec1, den0)
                cnv1 = wk_.tile([64, SC], BF, name=f"cnv1_{c}_{u}",
                                tag="cnv1", bufs=2)
                nc.vector.tensor_copy(cnv1, cps[1][0:64, :])
                bc1 = wk_.tile([64, SC], F32, name=f"bc1_{c}_{u}",
                               tag="bc1", bufs=2)
                nc.gpsimd.partition_broadcast(bc1, rec1)
                nc.vector.tensor_mul(cts[u][0:64, cs], cnv1, bc1)

        def emit_outproj(c):
            for mi in range(4):
                m = 4 * c + mi
                mb = slice(m * 128, (m + 1) * 128)
                for n in range(4):
                    nck = slice(n * SC, (n + 1) * SC)
                    po = pp.tile([128, SC], F32, name=f"po_{c}_{mi}_{n}", tag="po")
                    for u in range(2):
                        nc.tensor.matmul(
                            po,
                            lhsT=cts[u][:, mb],
                            rhs=wo_sb[:, u, nck],
                            start=(u == 0), stop=(u == 1),
                        )
                    ob = wk_.tile([128, SC], BF, name=f"ob_{c}_{mi}_{n}",
                                  tag="ob", bufs=2)
                    nc.vector.tensor_copy(ob, po)
                    nc.sync.dma_start(out=out[mb, nck], in_=ob)

        emit_proj(0)
        for c in range(NSC):
            emit_attn(c)
            if c + 1 < NSC:
                emit_proj(c + 1)
            emit_outproj(c)

    nc.finalize()
    return nc


def _get_nc():
    global _NC
    if _NC is None:
        _NC = _build()
    return _NC


def _prep_in_maps(x, Wq, Wk, Wv, Wo, cos, sin):
    BFNP = mybir.dt.np(BF)
    x0 = np.ascontiguousarray(np.asarray(x, np.float32).reshape(S, D))
    xT = np.ascontiguousarray(
        x0.T.reshape(KB, 128, S).transpose(1, 0, 2)).astype(BFNP)
    cosT = np.ascontiguousarray(np.asarray(cos, np.float32).T)  # (32, S)
    sinT = np.ascontiguousarray(np.asarray(sin, np.float32).T)
    ctab = np.tile(cosT, (4, 1)).astype(BFNP)                     # (128, S)
    # pre-swapped sign table: rope = q*C + swap32(q*S2)
    stab2 = np.tile(np.vstack([sinT, -sinT]), (2, 1)).astype(BFNP)
    trimask = (np.arange(128)[:, None] <= np.arange(128)[None, :]).astype(
        np.float32).astype(BFNP)
    eye = np.eye(64, dtype=np.float32)
    vones_a = np.zeros((128, KB, 2), np.float32)
    vones_a[:, :, 0] = 1.0
    vones_a = vones_a.astype(BFNP)
    vpad_a = np.zeros((128, KB, 63), np.float32).astype(BFNP)
    Wq = np.asarray(Wq, np.float32)
    Wk = np.asarray(Wk, np.float32)
    Wv = np.asarray(Wv, np.float32)
    Wo = np.asarray(Wo, np.float32)

    in_maps = []
    for i in range(NCORES):
        wq_i = np.ascontiguousarray(
            Wq[:, i * QC:(i + 1) * QC].reshape(KB, 128, QC).transpose(1, 0, 2)
        ).astype(BFNP)
        wkv_i = np.concatenate(
            [Wk[:, i * HD:(i + 1) * HD], Wv[:, i * HD:(i + 1) * HD]], axis=1)
        wkv_i = np.ascontiguousarray(
            wkv_i.reshape(KB, 128, 128).transpose(1, 0, 2)).astype(BFNP)
        wo_blocks = Wo[i * QC:(i + 1) * QC, :].reshape(2, 128, D)
        # cts rows 0-63 hold head 2u+1, rows 64-127 head 2u: swap halves
        wo_blocks = np.concatenate(
            [wo_blocks[:, 64:128, :], wo_blocks[:, 0:64, :]], axis=1)
        wo_i = np.ascontiguousarray(
            wo_blocks.transpose(1, 0, 2)).astype(BFNP)
        in_maps.append({
            "xT": xT, "wq": wq_i, "wkv": wkv_i, "wo": wo_i,
            "ctab": ctab, "stab2": stab2, "trimask": trimask, "eye": eye,
            "vones": vones_a, "vpad": vpad_a,
        })
    return in_maps


def run(inputs, **kw):
    nc = _get_nc()
    in_maps = _prep_in_maps(**inputs)
    return run_bass_kernel_spmd(nc, in_maps, list(range(NCORES)), **kw)


def kernel(x, Wq, Wk, Wv, Wo, cos, sin):
    res = run(dict(x=x, Wq=Wq, Wk=Wk, Wv=Wv, Wo=Wo, cos=cos, sin=sin))
    acc = np.zeros((S, D), np.float32)
    for r in res.results:
        acc += np.asarray(r["out"], np.float32)
    return acc.reshape(1, S, D)


# revision 18
# speedup vs baseline: 1.6481x; 1.0433x over previous
"""GQA attention + RoPE, tensor-parallel across 8 NeuronCores (Bass/Tile).

Model: x(1,2048,2048) -> Q=xWq (32 heads x 64), K/V=xWk/xWv (8 kv heads),
RoPE on q/k, causal softmax attention (GQA: 4 q heads per kv head), out-proj.

Sharding: head-parallel. Core i gets q heads 4i..4i+3 (Wq cols), kv head i
(Wk/Wv cols), Wo rows 256i..256i+256. Each core computes a partial (2048,2048)
output; host sums the 8 partials (the "all-reduce").

v2 changes vs v1 (427us baseline):
  * All matmul operands bf16 (psum accum stays fp32): same PE rate as f32r
    but FWL halves LDWEIGHTS, DMA bytes halve, DVE gets 2x/4x modes.
  * RoPE without the ACT pre-copy: out = q*C + swap32(q*S2) with S2 a
    pre-swapped sign-adjusted sin table, so the DMA half-swap reads the
    DVE product directly (psum -> 2 DVE muls -> 4 small DMAs -> add).
  * Softmax normalize: reciprocal_approx_fast directly on the psum den row
    (was: 1-partition DMA + 3.3us DVE reciprocal), gpsimd broadcast from
    partition 64, single mul. No den DMA.
  * Causal handling: ctx matmuls restrict rhs/out to the live column range
    instead of zero-padding e (drops the zpad DMAs).
  * xc streamed as 4 DMAs of 4 k-blocks; emission order attn(c), proj(c+1),
    outproj(c) so ready proj/outproj matmuls fill PE stalls.
"""

import numpy as np
from contextlib import ExitStack

import concourse.bass as bass
from concourse import bacc
import concourse.tile as tile
from concourse import mybir
from concourse.bass_utils import run_bass_kernel_spmd

F32 = mybir.dt.float32
BF = mybir.dt.bfloat16
AF = mybir.ActivationFunctionType

S = 2048          # sequence length
D = 2048          # model dim
HD = 64           # head dim
NCORES = 8
QH = 4            # q heads per core
QC = QH * HD      # 256 q columns per core
SC = 512          # seq chunk width
NSC = S // SC     # 4 chunks
KB = D // 128     # 16 feature blocks
SCALE = 1.0 / 8.0  # 1/sqrt(64)

_NC = None


def _build():
    nc = bacc.Bacc(None)
    xT = nc.declare_dram_parameter("xT", [128, KB, S], BF, isOutput=False)
    wq = nc.declare_dram_parameter("wq", [128, KB, QC], BF, isOutput=False)
    wkv = nc.declare_dram_parameter("wkv", [128, KB, 128], BF, isOutput=False)
    wo = nc.declare_dram_parameter("wo", [128, 2, D], BF, isOutput=False)
    ctab = nc.declare_dram_parameter("ctab", [128, S], BF, isOutput=False)
    stab2 = nc.declare_dram_parameter("stab2", [128, S], BF, isOutput=False)
    trimask = nc.declare_dram_parameter("trimask", [128, 128], BF, isOutput=False)
    eye = nc.declare_dram_parameter("eye", [64, 64], F32, isOutput=False)
    vones = nc.declare_dram_parameter("vones", [128, KB, 2], BF, isOutput=False)
    vpad = nc.declare_dram_parameter("vpad", [128, KB, 63], BF, isOutput=False)
    out = nc.declare_dram_parameter("out", [S, D], BF, isOutput=True)

    with tile.TileContext(nc) as tc, ExitStack() as ctx:
        sb = ctx.enter_context(tc.tile_pool(name="sb", bufs=1))
        xp = ctx.enter_context(tc.tile_pool(name="xp", bufs=2))
        wk_ = ctx.enter_context(tc.tile_pool(name="wk", bufs=2))
        pp = ctx.enter_context(tc.tile_pool(name="pp", bufs=2, space="PSUM"))

        # ---- persistent constants (ordered so chunk-0 Q matmuls + rope can
        # start ASAP; wo & attn-only tables load in the background) ----
        wq_sb = sb.tile([128, KB, QC], BF)
        nc.sync.dma_start(out=wq_sb[:, 0:4, :], in_=wq[:, 0:4, :])
        ctab_sb = sb.tile([128, S], BF)
        nc.sync.dma_start(out=ctab_sb, in_=ctab[:, :])
        stab_sb = sb.tile([128, S], BF)
        nc.sync.dma_start(out=stab_sb, in_=stab2[:, :])
        nc.sync.dma_start(out=wq_sb[:, 4:KB, :], in_=wq[:, 4:KB, :])
        wkv_sb = sb.tile([128, KB, 128], BF)
        nc.sync.dma_start(out=wkv_sb, in_=wkv[:, :, :])
        tri_sb = sb.tile([128, 128], BF)
        nc.gpsimd.dma_start(out=tri_sb, in_=trimask[:, :])
        eye_sb = sb.tile([64, 64], F32)
        nc.gpsimd.dma_start(out=eye_sb, in_=eye[:, :])
        wo_sb = sb.tile([128, 2, D], BF)
        nc.gpsimd.dma_start(out=wo_sb, in_=wo[:, :, :])

        # ---- persistent activations ----
        qt0 = sb.tile([128, S], BF)   # q^T heads 0,1 (roped)
        qt1 = sb.tile([128, S], BF)   # q^T heads 2,3
        qts = [qt0, qt1]
        kt_sb = sb.tile([128, S], BF)  # rows 0-63 k^T roped; 64-127 duplicate
        v_sb = sb.tile([128, KB, HD + 2], BF)  # [V | ones | pad] (for idx1)
        v_sb2 = sb.tile([128, KB, 128], BF)    # [ones | zeros*63 | V] (idx0)
        ct0 = sb.tile([128, S], BF)   # normalized ctx^T: rows 0-63 head 2u+1,
        ct1 = sb.tile([128, S], BF)   # rows 64-127 head 2u (wo host-reordered)
        cts = [ct0, ct1]
        nc.gpsimd.dma_start(out=v_sb[:, :, HD:HD + 2], in_=vones[:, :, :])
        nc.gpsimd.dma_start(out=v_sb2[:, :, 0:1], in_=vones[:, :, 0:1])
        nc.gpsimd.dma_start(out=v_sb2[:, :, 1:64], in_=vpad[:, :, :])

        def emit_proj(c):
            cs = slice(c * SC, (c + 1) * SC)
            xc = xp.tile([128, KB, SC], BF, name=f"xc_{c}", tag="xc")
            for g in range(4):
                nc.sync.dma_start(out=xc[:, 4 * g:4 * g + 4, :],
                                  in_=xT[:, 4 * g:4 * g + 4, cs])
            # Q projection + rope, one 128-tile (2 heads) at a time; the
            # sin-product halves of both u-tiles share one tile so the
            # 32-row half-swap costs 4 DMAs per chunk instead of 8
            t1s = []
            w = wk_.tile([128, 2, SC], BF, name=f"rw_{c}", tag="rw", bufs=2)
            for u in range(2):
                pq = pp.tile([128, SC], F32, name=f"pq_{c}_{u}", tag="pq")
                for kb in range(KB):
                    nc.tensor.matmul(
                        pq,
                        lhsT=wq_sb[:, kb, u * 128:(u + 1) * 128],
                        rhs=xc[:, kb, :],
                        start=(kb == 0), stop=(kb == KB - 1),
                    )
                t1 = wk_.tile([128, SC], BF, name=f"rt1_{c}_{u}", tag="rt",
                              bufs=3)
                nc.vector.tensor_mul(t1, pq, ctab_sb[:, cs])
                t1s.append(t1)
                nc.vector.tensor_mul(w[:, u, :], pq, stab_sb[:, cs])
            wsw = wk_.tile([128, 2, SC], BF, name=f"wsw_{c}", tag="wsw",
                           bufs=2)
            for b in (0, 64):
                nc.sync.dma_start(out=wsw[b:b + 32, :, :],
                                  in_=w[b + 32:b + 64, :, :])
                nc.sync.dma_start(out=wsw[b + 32:b + 64, :, :],
                                  in_=w[b:b + 32, :, :])
            for u in range(2):
                nc.vector.tensor_add(qts[u][:, cs], t1s[u], wsw[:, u, :])
            # KV projection
            pkv = pp.tile([128, SC], F32, name=f"pkv_{c}", tag="pq")
            for kb in range(KB):
                nc.tensor.matmul(
                    pkv,
                    lhsT=wkv_sb[:, kb, :],
                    rhs=xc[:, kb, :],
                    start=(kb == 0), stop=(kb == KB - 1),
                )
            # K rope (rows 0-63)
            k1 = wk_.tile([64, SC], BF, name=f"kr1_{c}", tag="krt", bufs=2)
            nc.vector.tensor_mul(k1, pkv[0:64, :], ctab_sb[0:64, cs])
            kw = wk_.tile([64, SC], BF, name=f"krw_{c}", tag="krw", bufs=2)
            nc.vector.tensor_mul(kw, pkv[0:64, :], stab_sb[0:64, cs])
            ksw = wk_.tile([64, SC], BF, name=f"ksw_{c}", tag="ksw")
            nc.sync.dma_start(out=ksw[0:32, :], in_=kw[32:64, :])
            nc.sync.dma_start(out=ksw[32:64, :], in_=kw[0:32, :])
            nc.vector.tensor_add(kt_sb[0:64, cs], k1, ksw)
            nc.sync.dma_start(out=kt_sb[64:128, cs], in_=kt_sb[0:64, cs])
            # V natural layout: copy psum rows 64-127 (fp32), shift down via
            # DMA, then PE-transpose each 128-seq block
            vraw = wk_.tile([128, SC], F32, name=f"vraw_{c}", tag="vraw")
            nc.vector.tensor_copy(vraw[64:128, :], pkv[64:128, :])
            vtr = wk_.tile([64, SC], F32, name=f"vtr_{c}", tag="vtr")
            nc.sync.dma_start(out=vtr, in_=vraw[64:128, :])
            for r in range(4):
                j = 4 * c + r
                pt = pp.tile([128, HD], F32, name=f"pt_{c}_{r}", tag="ps")
                nc.tensor.transpose(pt, vtr[:, r * 128:(r + 1) * 128], eye_sb)
                nc.vector.tensor_copy(v_sb[:, j, 0:HD], pt)
                nc.vector.tensor_copy(v_sb2[:, j, HD:2 * HD], pt)

        def emit_attn(c):
            cs = slice(c * SC, (c + 1) * SC)
            for u in range(2):
                # idx0: lhsT=[ones|0*63|V] -> den row 0, ctx rows 64-127
                # idx1: lhsT=[V|ones|pad]  -> ctx rows 0-63, den row 64
                cps = [
                    pp.tile([128, SC], F32, name=f"cp_{c}_{u}_0", tag="pc"),
                    pp.tile([HD + 2, SC], F32, name=f"cp_{c}_{u}_1", tag="pc"),
                ]
                njt = 4 * c + 4
                for j in range(njt):
                    diag = j >= 4 * c
                    r = j - 4 * c
                    jb = slice(j * 128, (j + 1) * 128)
                    # columns of this chunk still unmasked for t-block j
                    lo = 128 * r if diag else 0
                    nsl = slice(lo, SC)
                    csl = slice(c * SC + lo, (c + 1) * SC)
                    for idx in range(2):
                        sp = pp.tile([128, SC], F32, name=f"sp_{c}_{u}_{j}_{idx}",
                                     tag="ps")
                        nc.tensor.matmul(
                            sp[:, nsl],
                            lhsT=kt_sb[idx * 64:idx * 64 + 64, jb],
                            rhs=qts[u][idx * 64:idx * 64 + 64, csl],
                            start=True, stop=True,
                            tile_position=(idx * 64, 0),
                        )
                        e = wk_.tile([128, SC], BF, name=f"e_{c}_{u}_{j}_{idx}",
                                     tag="ex", bufs=3)
                        nc.scalar.activation(e[:, nsl], sp[:, nsl], AF.Exp,
                                             scale=SCALE)
                        if diag:
                            dsl = slice(lo, lo + 128)
                            nc.vector.tensor_mul(e[:, dsl], e[:, dsl], tri_sb)
                        nc.tensor.matmul(
                            cps[idx][:, nsl],
                            lhsT=(v_sb2[:, j, :] if idx == 0
                                  else v_sb[:, j, :]),
                            rhs=e[:, nsl],
                            start=(j == 0), stop=(j == njt - 1),
                            skip_group_check=True,
                        )
                # idx0: den at psum row 0 -> reciprocal -> broadcast (from
                # partition 0) to all 128 partitions; ctx rows 64-127 scale
                # in place into cts rows 64-127. No cross-partition DMA.
                rec0 = wk_.tile([1, SC], F32, name=f"rec0_{c}_{u}",
                                tag="rec0", bufs=2)
                nc.vector.reciprocal_approx_fast(rec0, cps[0][0:1, :])
                cnv0 = wk_.tile([128, SC], BF, name=f"cnv0_{c}_{u}",
                                tag="cnv0", bufs=2)
                nc.vector.tensor_copy(cnv0[64:128, :], cps[0][64:128, :])
                bc0 = wk_.tile([128, SC], F32, name=f"bc0_{c}_{u}",
                               tag="bc0", bufs=2)
                nc.gpsimd.partition_broadcast(bc0, rec0)
                nc.vector.tensor_mul(cts[u][64:128, cs], cnv0[64:128, :],
                                     bc0[64:128, :])
                # idx1: den at psum row 64 -> copy to sbuf, DMA to partition
                # 0, reciprocal there, broadcast, scale rows 0-63.
                dcp = wk_.tile([66, SC], F32, name=f"dcp_{c}_{u}",
                               tag="dcp", bufs=2)
                nc.vector.tensor_copy(dcp[64:65, :], cps[1][64:65, :])
                den0 = wk_.tile([1, SC], F32, name=f"den0_{c}_{u}",
                                tag="den0", bufs=2)
                nc.sync.dma_start(out=den0, in_=dcp[64:65, :])
                rec1 = wk_.tile([1, SC], F32, name=f"rec1_{c}_{u}",
                                tag="rec1", bufs=2)
                nc.vector.reciprocal_approx_fast(rec1, den0)
                cnv1 = wk_.tile([64, SC], BF, name=f"cnv1_{c}_{u}",
                                tag="cnv1", bufs=2)
                nc.vector.tensor_copy(cnv1, cps[1][0:64, :])
                bc1 = wk_.tile([64, SC], F32, name=f"bc1_{c}_{u}",
                               tag="bc1", bufs=2)
                nc.gpsimd.partition_broadcast(bc1, rec1)
                nc.vector.tensor_mul(cts[u][0:64, cs], cnv1, bc1)

        def emit_outproj(c):
            for mi in range(4):
                m = 4 * c + mi
                mb = slice(m * 128, (m + 1) * 128)
                ob = wk_.tile([128, D], BF, name=f"ob_{c}_{mi}",
                              tag="ob", bufs=2)
                for n in range(4):
                    nck = slice(n * SC, (n + 1) * SC)
                    po = pp.tile([128, SC], F32, name=f"po_{c}_{mi}_{n}", tag="po")
                    for u in range(2):
                        nc.tensor.matmul(
                            po,
                            lhsT=cts[u][:, mb],
                            rhs=wo_sb[:, u, nck],
                            start=(u == 0), stop=(u == 1),
                        )
                    nc.vector.tensor_copy(ob[:, nck], po)
                nc.gpsimd.dma_start(out=out[mb, :], in_=ob)

        emit_proj(0)
        for c in range(NSC):
            emit_attn(c)
            if c + 1 < NSC:
                emit_proj(c + 1)
            emit_outproj(c)

    nc.finalize()
    return nc


def _get_nc():
    global _NC
    if _NC is None:
        _NC = _build()
    return _NC


def _prep_in_maps(x, Wq, Wk, Wv, Wo, cos, sin):
    BFNP = mybir.dt.np(BF)
    x0 = np.ascontiguousarray(np.asarray(x, np.float32).reshape(S, D))
    xT = np.ascontiguousarray(
        x0.T.reshape(KB, 128, S).transpose(1, 0, 2)).astype(BFNP)
    cosT = np.ascontiguousarray(np.asarray(cos, np.float32).T)  # (32, S)
    sinT = np.ascontiguousarray(np.asarray(sin, np.float32).T)
    ctab = np.tile(cosT, (4, 1)).astype(BFNP)                     # (128, S)
    # pre-swapped sign table: rope = q*C + swap32(q*S2)
    stab2 = np.tile(np.vstack([sinT, -sinT]), (2, 1)).astype(BFNP)
    trimask = (np.arange(128)[:, None] <= np.arange(128)[None, :]).astype(
        np.float32).astype(BFNP)
    eye = np.eye(64, dtype=np.float32)
    vones_a = np.zeros((128, KB, 2), np.float32)
    vones_a[:, :, 0] = 1.0
    vones_a = vones_a.astype(BFNP)
    vpad_a = np.zeros((128, KB, 63), np.float32).astype(BFNP)
    Wq = np.asarray(Wq, np.float32)
    Wk = np.asarray(Wk, np.float32)
    Wv = np.asarray(Wv, np.float32)
    Wo = np.asarray(Wo, np.float32)

    in_maps = []
    for i in range(NCORES):
        wq_i = np.ascontiguousarray(
            Wq[:, i * QC:(i + 1) * QC].reshape(KB, 128, QC).transpose(1, 0, 2)
        ).astype(BFNP)
        wkv_i = np.concatenate(
            [Wk[:, i * HD:(i + 1) * HD], Wv[:, i * HD:(i + 1) * HD]], axis=1)
        wkv_i = np.ascontiguousarray(
            wkv_i.reshape(KB, 128, 128).transpose(1, 0, 2)).astype(BFNP)
        wo_blocks = Wo[i * QC:(i + 1) * QC, :].reshape(2, 128, D)
        # cts rows 0-63 hold head 2u+1, rows 64-127 head 2u: swap halves
        wo_blocks = np.concatenate(
            [wo_blocks[:, 64:128, :], wo_blocks[:, 0:64, :]], axis=1)
        wo_i = np.ascontiguousarray(
            wo_blocks.transpose(1, 0, 2)).astype(BFNP)
        in_maps.append({
            "xT": xT, "wq": wq_i, "wkv": wkv_i, "wo": wo_i,
            "ctab": ctab, "stab2": stab2, "trimask": trimask, "eye": eye,
            "vones": vones_a, "vpad": vpad_a,
        })
    return in_maps


def run(inputs, **kw):
    nc = _get_nc()
    in_maps = _prep_in_maps(**inputs)
    return run_bass_kernel_spmd(nc, in_maps, list(range(NCORES)), **kw)


def kernel(x, Wq, Wk, Wv, Wo, cos, sin):
    res = run(dict(x=x, Wq=Wq, Wk=Wk, Wv=Wv, Wo=Wo, cos=cos, sin=sin))
    acc = np.zeros((S, D), np.float32)
    for r in res.results:
        acc += np.asarray(r["out"], np.float32)
    return acc.reshape(1, S, D)


# revision 43
# speedup vs baseline: 1.8722x; 1.1360x over previous
"""GQA attention + RoPE, tensor-parallel across 8 NeuronCores (Bass/Tile).

Model: x(1,2048,2048) -> Q=xWq (32 heads x 64), K/V=xWk/xWv (8 kv heads),
RoPE on q/k, causal softmax attention (GQA: 4 q heads per kv head), out-proj.

Sharding: head-parallel. Core i gets q heads 4i..4i+3 (Wq cols), kv head i
(Wk/Wv cols), Wo rows 256i..256i+256. Each core computes a partial (2048,2048)
output; host sums the 8 partials (the "all-reduce").

v2 changes vs v1 (427us baseline):
  * All matmul operands bf16 (psum accum stays fp32): same PE rate as f32r
    but FWL halves LDWEIGHTS, DMA bytes halve, DVE gets 2x/4x modes.
  * RoPE without the ACT pre-copy: out = q*C + swap32(q*S2) with S2 a
    pre-swapped sign-adjusted sin table, so the DMA half-swap reads the
    DVE product directly (psum -> 2 DVE muls -> 4 small DMAs -> add).
  * Softmax normalize: reciprocal_approx_fast directly on the psum den row
    (was: 1-partition DMA + 3.3us DVE reciprocal), gpsimd broadcast from
    partition 64, single mul. No den DMA.
  * Causal handling: ctx matmuls restrict rhs/out to the live column range
    instead of zero-padding e (drops the zpad DMAs).
  * xc streamed as 4 DMAs of 4 k-blocks; emission order attn(c), proj(c+1),
    outproj(c) so ready proj/outproj matmuls fill PE stalls.
"""

import numpy as np
from contextlib import ExitStack

import concourse.bass as bass
from concourse import bacc
import concourse.tile as tile
from concourse import mybir
from concourse.bass_utils import run_bass_kernel_spmd

F32 = mybir.dt.float32
BF = mybir.dt.bfloat16
AF = mybir.ActivationFunctionType

S = 2048          # sequence length
D = 2048          # model dim
HD = 64           # head dim
NCORES = 8
QH = 4            # q heads per core
QC = QH * HD      # 256 q columns per core
SC = 512          # seq chunk width
NSC = S // SC     # 4 chunks
KB = D // 128     # 16 feature blocks
SCALE = 1.0 / 8.0  # 1/sqrt(64)

_NC = None


def _build():
    nc = bacc.Bacc(None)
    xT = nc.declare_dram_parameter("xT", [128, KB, S], BF, isOutput=False)
    wq = nc.declare_dram_parameter("wq", [128, KB, QC], BF, isOutput=False)
    wkv = nc.declare_dram_parameter("wkv", [128, KB, 128], BF, isOutput=False)
    wo = nc.declare_dram_parameter("wo", [128, 2, D], BF, isOutput=False)
    ctab = nc.declare_dram_parameter("ctab", [128, S], BF, isOutput=False)
    stab2 = nc.declare_dram_parameter("stab2", [128, S], BF, isOutput=False)
    trimask = nc.declare_dram_parameter("trimask", [128, 128], BF, isOutput=False)
    eye = nc.declare_dram_parameter("eye", [64, 64], F32, isOutput=False)
    vones = nc.declare_dram_parameter("vones", [128, KB, 2], BF, isOutput=False)
    vpad = nc.declare_dram_parameter("vpad", [128, KB, 63], BF, isOutput=False)
    out = nc.declare_dram_parameter("out", [S, D], BF, isOutput=True)

    with tile.TileContext(nc) as tc, ExitStack() as ctx:
        sb = ctx.enter_context(tc.tile_pool(name="sb", bufs=1))
        xp = ctx.enter_context(tc.tile_pool(name="xp", bufs=2))
        wk_ = ctx.enter_context(tc.tile_pool(name="wk", bufs=2))
        pp = ctx.enter_context(tc.tile_pool(name="pp", bufs=2, space="PSUM"))

        # ---- persistent constants (ordered so chunk-0 Q matmuls + rope can
        # start ASAP; wo & attn-only tables load in the background) ----
        wq_sb = sb.tile([128, KB, QC], BF)
        nc.sync.dma_start(out=wq_sb[:, 0:4, :], in_=wq[:, 0:4, :])
        ctab_sb = sb.tile([128, S], BF)
        nc.sync.dma_start(out=ctab_sb, in_=ctab[:, :])
        stab_sb = sb.tile([128, S], BF)
        nc.sync.dma_start(out=stab_sb, in_=stab2[:, :])
        nc.sync.dma_start(out=wq_sb[:, 4:KB, :], in_=wq[:, 4:KB, :])
        wkv_sb = sb.tile([128, KB, 128], BF)
        nc.sync.dma_start(out=wkv_sb, in_=wkv[:, :, :])
        tri_sb = sb.tile([128, 128], BF)
        nc.gpsimd.dma_start(out=tri_sb, in_=trimask[:, :])
        eye_sb = sb.tile([64, 64], F32)
        nc.gpsimd.dma_start(out=eye_sb, in_=eye[:, :])
        wo_sb = sb.tile([128, 2, D], BF)
        nc.gpsimd.dma_start(out=wo_sb, in_=wo[:, :, :])

        # ---- persistent activations ----
        qt0 = sb.tile([128, S], BF)   # q^T heads 0,1 (roped)
        qt1 = sb.tile([128, S], BF)   # q^T heads 2,3
        qts = [qt0, qt1]
        kt_sb = sb.tile([128, S], BF)  # rows 0-63 k^T roped; 64-127 duplicate
        v_sb = sb.tile([128, KB, HD + 2], BF)  # [V | ones | pad] (for idx1)
        v_sb2 = sb.tile([128, KB, 128], BF)    # [ones | zeros*63 | V] (idx0)
        ct0 = sb.tile([128, S], BF)   # normalized ctx^T: rows 0-63 head 2u+1,
        ct1 = sb.tile([128, S], BF)   # rows 64-127 head 2u (wo host-reordered)
        cts = [ct0, ct1]
        nc.gpsimd.dma_start(out=v_sb[:, :, HD:HD + 2], in_=vones[:, :, :])
        nc.gpsimd.dma_start(out=v_sb2[:, :, 0:1], in_=vones[:, :, 0:1])
        nc.gpsimd.dma_start(out=v_sb2[:, :, 1:64], in_=vpad[:, :, :])

        def proj_gen(c):
            """Projection for chunk c as 3 interleavable units (yield after
            each): Q(u0)+rope-muls / Q(u1)+swap+adds / KV+k-rope+V."""
            cs = slice(c * SC, (c + 1) * SC)
            xc = xp.tile([128, KB, SC], BF, name=f"xc_{c}", tag="xc")
            for g in range(4):
                nc.sync.dma_start(out=xc[:, 4 * g:4 * g + 4, :],
                                  in_=xT[:, 4 * g:4 * g + 4, cs])
            # Q projection + rope, one 128-tile (2 heads) at a time; the
            # sin-product halves of both u-tiles share one tile so the
            # 32-row half-swap costs 4 DMAs per chunk instead of 8
            t1s = []
            w = wk_.tile([128, 2, SC], BF, name=f"rw_{c}", tag="rw", bufs=2)
            for u in range(2):
                pq = pp.tile([128, SC], F32, name=f"pq_{c}_{u}", tag="pq")
                for kb in range(KB):
                    nc.tensor.matmul(
                        pq,
                        lhsT=wq_sb[:, kb, u * 128:(u + 1) * 128],
                        rhs=xc[:, kb, :],
                        start=(kb == 0), stop=(kb == KB - 1),
                    )
                t1 = wk_.tile([128, SC], BF, name=f"rt1_{c}_{u}", tag="rt",
                              bufs=3)
                nc.vector.tensor_mul(t1, pq, ctab_sb[:, cs])
                t1s.append(t1)
                nc.vector.tensor_mul(w[:, u, :], pq, stab_sb[:, cs])
                if u == 0:
                    yield
            wsw = wk_.tile([128, 2, SC], BF, name=f"wsw_{c}", tag="wsw",
                           bufs=2)
            for b in (0, 64):
                nc.sync.dma_start(out=wsw[b:b + 32, :, :],
                                  in_=w[b + 32:b + 64, :, :])
                nc.sync.dma_start(out=wsw[b + 32:b + 64, :, :],
                                  in_=w[b:b + 32, :, :])
            for u in range(2):
                nc.vector.tensor_add(qts[u][:, cs], t1s[u], wsw[:, u, :])
            yield
            # KV projection
            pkv = pp.tile([128, SC], F32, name=f"pkv_{c}", tag="pq")
            for kb in range(KB):
                nc.tensor.matmul(
                    pkv,
                    lhsT=wkv_sb[:, kb, :],
                    rhs=xc[:, kb, :],
                    start=(kb == 0), stop=(kb == KB - 1),
                )
            # K rope (rows 0-63)
            k1 = wk_.tile([64, SC], BF, name=f"kr1_{c}", tag="krt", bufs=2)
            nc.vector.tensor_mul(k1, pkv[0:64, :], ctab_sb[0:64, cs])
            kw = wk_.tile([64, SC], BF, name=f"krw_{c}", tag="krw", bufs=2)
            nc.vector.tensor_mul(kw, pkv[0:64, :], stab_sb[0:64, cs])
            ksw = wk_.tile([64, SC], BF, name=f"ksw_{c}", tag="ksw")
            nc.sync.dma_start(out=ksw[0:32, :], in_=kw[32:64, :])
            nc.sync.dma_start(out=ksw[32:64, :], in_=kw[0:32, :])
            nc.vector.tensor_add(kt_sb[0:64, cs], k1, ksw)
            nc.sync.dma_start(out=kt_sb[64:128, cs], in_=kt_sb[0:64, cs])
            # V natural layout: copy psum rows 64-127 (fp32), shift down via
            # DMA, then PE-transpose each 128-seq block
            vraw = wk_.tile([128, SC], F32, name=f"vraw_{c}", tag="vraw")
            nc.vector.tensor_copy(vraw[64:128, :], pkv[64:128, :])
            vtr = wk_.tile([64, SC], F32, name=f"vtr_{c}", tag="vtr")
            nc.sync.dma_start(out=vtr, in_=vraw[64:128, :])
            for r in range(4):
                j = 4 * c + r
                pt = pp.tile([128, HD], F32, name=f"pt_{c}_{r}", tag="ps")
                nc.tensor.transpose(pt, vtr[:, r * 128:(r + 1) * 128], eye_sb)
                nc.vector.tensor_copy(v_sb[:, j, 0:HD], pt)
                nc.vector.tensor_copy(v_sb2[:, j, HD:2 * HD], pt)
            yield

        def emit_attn(c, units):
            # `units` are ready-to-run generator steps (outproj of chunk c-1,
            # proj of chunk c+1) spread through the j-loop so the PE queue
            # always holds independent matmuls behind exp-gated ctx matmuls.
            cs = slice(c * SC, (c + 1) * SC)
            nslots = 2 * (4 * c + 4)
            quota, acc, ui = len(units) / nslots, 0.0, 0

            def fill():
                nonlocal acc, ui
                acc += quota
                while acc >= 1.0 and ui < len(units):
                    units[ui]()
                    ui += 1
                    acc -= 1.0

            for u in range(2):
                # idx0: lhsT=[ones|0*63|V] -> den row 0, ctx rows 64-127
                # idx1: lhsT=[V|ones|pad]  -> ctx rows 0-63, den row 64
                cps = [
                    pp.tile([128, SC], F32, name=f"cp_{c}_{u}_0", tag="pc"),
                    pp.tile([HD + 2, SC], F32, name=f"cp_{c}_{u}_1", tag="pc"),
                ]
                njt = 4 * c + 4
                for j in range(njt):
                    diag = j >= 4 * c
                    r = j - 4 * c
                    jb = slice(j * 128, (j + 1) * 128)
                    # columns of this chunk still unmasked for t-block j
                    lo = 128 * r if diag else 0
                    nsl = slice(lo, SC)
                    csl = slice(c * SC + lo, (c + 1) * SC)
                    for idx in range(2):
                        sp = pp.tile([128, SC], F32, name=f"sp_{c}_{u}_{j}_{idx}",
                                     tag="ps")
                        nc.tensor.matmul(
                            sp[:, nsl],
                            lhsT=kt_sb[idx * 64:idx * 64 + 64, jb],
                            rhs=qts[u][idx * 64:idx * 64 + 64, csl],
                            start=True, stop=True,
                            tile_position=(idx * 64, 0),
                        )
                        e = wk_.tile([128, SC], BF, name=f"e_{c}_{u}_{j}_{idx}",
                                     tag="ex", bufs=6)
                        nc.scalar.activation(e[:, nsl], sp[:, nsl], AF.Exp,
                                             scale=SCALE)
                        if diag:
                            dsl = slice(lo, lo + 128)
                            nc.vector.tensor_mul(e[:, dsl], e[:, dsl], tri_sb)
                        nc.tensor.matmul(
                            cps[idx][:, nsl],
                            lhsT=(v_sb2[:, j, :] if idx == 0
                                  else v_sb[:, j, :]),
                            rhs=e[:, nsl],
                            start=(j == 0), stop=(j == njt - 1),
                            skip_group_check=True,
                        )
                    fill()
                # idx0: den at psum row 0 -> reciprocal -> broadcast (from
                # partition 0) to all 128 partitions; ctx rows 64-127 scale
                # in place into cts rows 64-127. No cross-partition DMA.
                rec0 = wk_.tile([1, SC], F32, name=f"rec0_{c}_{u}",
                                tag="rec0", bufs=2)
                nc.vector.reciprocal_approx_fast(rec0, cps[0][0:1, :])
                cnv0 = wk_.tile([128, SC], BF, name=f"cnv0_{c}_{u}",
                                tag="cnv0", bufs=2)
                nc.vector.tensor_copy(cnv0[64:128, :], cps[0][64:128, :])
                bc0 = wk_.tile([128, SC], F32, name=f"bc0_{c}_{u}",
                               tag="bc0", bufs=2)
                nc.gpsimd.partition_broadcast(bc0, rec0)
                nc.vector.tensor_mul(cts[u][64:128, cs], cnv0[64:128, :],
                                     bc0[64:128, :])
                # idx1: den at psum row 64 -> copy to sbuf, DMA to partition
                # 0, reciprocal there, broadcast, scale rows 0-63.
                dcp = wk_.tile([66, SC], F32, name=f"dcp_{c}_{u}",
                               tag="dcp", bufs=2)
                nc.vector.tensor_copy(dcp[64:65, :], cps[1][64:65, :])
                den0 = wk_.tile([1, SC], F32, name=f"den0_{c}_{u}",
                                tag="den0", bufs=2)
                nc.sync.dma_start(out=den0, in_=dcp[64:65, :])
                rec1 = wk_.tile([1, SC], F32, name=f"rec1_{c}_{u}",
                                tag="rec1", bufs=2)
                nc.vector.reciprocal_approx_fast(rec1, den0)
                cnv1 = wk_.tile([64, SC], BF, name=f"cnv1_{c}_{u}",
                                tag="cnv1", bufs=2)
                nc.vector.tensor_copy(cnv1, cps[1][0:64, :])
                bc1 = wk_.tile([64, SC], F32, name=f"bc1_{c}_{u}",
                               tag="bc1", bufs=2)
                nc.gpsimd.partition_broadcast(bc1, rec1)
                nc.vector.tensor_mul(cts[u][0:64, cs], cnv1, bc1)

        def outproj_gen(c):
            """Out-projection for chunk c as 16 interleavable (mi, n) units."""
            for mi in range(4):
                m = 4 * c + mi
                mb = slice(m * 128, (m + 1) * 128)
                ob = wk_.tile([128, D], BF, name=f"ob_{c}_{mi}",
                              tag="ob", bufs=2)
                for n in range(4):
                    nck = slice(n * SC, (n + 1) * SC)
                    po = pp.tile([128, SC], F32, name=f"po_{c}_{mi}_{n}", tag="po")
                    for u in range(2):
                        nc.tensor.matmul(
                            po,
                            lhsT=cts[u][:, mb],
                            rhs=wo_sb[:, u, nck],
                            start=(u == 0), stop=(u == 1),
                        )
                    nc.vector.tensor_copy(ob[:, nck], po)
                    if n == 3:
                        nc.gpsimd.dma_start(out=out[mb, :], in_=ob)
                    yield

        def drain(gen):
            for _ in gen:
                pass

        def step(gen):
            return lambda: next(gen, None)

        drain(proj_gen(0))
        for c in range(NSC):
            units = []
            if c >= 1:
                og = outproj_gen(c - 1)
                units += [step(og)] * 16
            if c + 1 < NSC:
                pg = proj_gen(c + 1)
                psteps = [step(pg)] * 3
                # place proj units at the 1/4, 1/2, 3/4 marks of the list
                if units:
                    units.insert(12, psteps[2])
                    units.insert(8, psteps[1])
                    units.insert(4, psteps[0])
                else:
                    units = psteps
            emit_attn(c, units)
            # any leftovers (rounding) run here, before the next chunk
            if c >= 1:
                drain(og)
            if c + 1 < NSC:
                drain(pg)
        drain(outproj_gen(NSC - 1))

    nc.finalize()
    return nc


def _get_nc():
    global _NC
    if _NC is None:
        _NC = _build()
    return _NC


def _prep_in_maps(x, Wq, Wk, Wv, Wo, cos, sin):
    BFNP = mybir.dt.np(BF)
    x0 = np.ascontiguousarray(np.asarray(x, np.float32).reshape(S, D))
    xT = np.ascontiguousarray(
        x0.T.reshape(KB, 128, S).transpose(1, 0, 2)).astype(BFNP)
    cosT = np.ascontiguousarray(np.asarray(cos, np.float32).T)  # (32, S)
    sinT = np.ascontiguousarray(np.asarray(sin, np.float32).T)
    ctab = np.tile(cosT, (4, 1)).astype(BFNP)                     # (128, S)
    # pre-swapped sign table: rope = q*C + swap32(q*S2)
    stab2 = np.tile(np.vstack([sinT, -sinT]), (2, 1)).astype(BFNP)
    trimask = (np.arange(128)[:, None] <= np.arange(128)[None, :]).astype(
        np.float32).astype(BFNP)
    eye = np.eye(64, dtype=np.float32)
    vones_a = np.zeros((128, KB, 2), np.float32)
    vones_a[:, :, 0] = 1.0
    vones_a = vones_a.astype(BFNP)
    vpad_a = np.zeros((128, KB, 63), np.float32).astype(BFNP)
    Wq = np.asarray(Wq, np.float32)
    Wk = np.asarray(Wk, np.float32)
    Wv = np.asarray(Wv, np.float32)
    Wo = np.asarray(Wo, np.float32)

    in_maps = []
    for i in range(NCORES):
        wq_i = np.ascontiguousarray(
            Wq[:, i * QC:(i + 1) * QC].reshape(KB, 128, QC).transpose(1, 0, 2)
        ).astype(BFNP)
        wkv_i = np.concatenate(
            [Wk[:, i * HD:(i + 1) * HD], Wv[:, i * HD:(i + 1) * HD]], axis=1)
        wkv_i = np.ascontiguousarray(
            wkv_i.reshape(KB, 128, 128).transpose(1, 0, 2)).astype(BFNP)
        wo_blocks = Wo[i * QC:(i + 1) * QC, :].reshape(2, 128, D)
        # cts rows 0-63 hold head 2u+1, rows 64-127 head 2u: swap halves
        wo_blocks = np.concatenate(
            [wo_blocks[:, 64:128, :], wo_blocks[:, 0:64, :]], axis=1)
        wo_i = np.ascontiguousarray(
            wo_blocks.transpose(1, 0, 2)).astype(BFNP)
        in_maps.append({
            "xT": xT, "wq": wq_i, "wkv": wkv_i, "wo": wo_i,
            "ctab": ctab, "stab2": stab2, "trimask": trimask, "eye": eye,
            "vones": vones_a, "vpad": vpad_a,
        })
    return in_maps


def run(inputs, **kw):
    nc = _get_nc()
    in_maps = _prep_in_maps(**inputs)
    return run_bass_kernel_spmd(nc, in_maps, list(range(NCORES)), **kw)


def kernel(x, Wq, Wk, Wv, Wo, cos, sin):
    res = run(dict(x=x, Wq=Wq, Wk=Wk, Wv=Wv, Wo=Wo, cos=cos, sin=sin))
    acc = np.zeros((S, D), np.float32)
    for r in res.results:
        acc += np.asarray(r["out"], np.float32)
    return acc.reshape(1, S, D)
